# revision 58
# baseline (speedup 1.0000x reference)
"""DiSAN Trainium2 Bass kernel — 8-core data parallel (one example per core).

Per-core layout (one batch example, both text blocks x1/x2):
  - m (key token, 128) on SBUF partitions; (i=query, d=feature) on free axis.
  - att pre-activation G[m, i, d] = h1[i,d]+b[d] + h2[m,d] is built on the
    TensorEngine: rank-1 broadcast of (h1+b) with a ones lhsT as an exact
    bf16 hi/lo pair accumulated in PSUM; h2 added by the Vector/GpSimd
    engines (alternating, to keep the DVE off the critical path).
  - ScalarEngine: A = tanh(G/c) fp32, z = exp(c*A) bf16 (Tanh/Exp share one
    activation-table set — no table switches). 16-query chunks amortize the
    per-instruction overhead.
  - TensorEngine: per-query "flipped" matmuls — lhsT = z (or z*h) slice
    [m=128, d-chunk], rhs = the query's fw/bw 0/1 bf16 mask columns
    [m=128, 2] — contract over m, producing the directional softmax sums S
    (denominator) and T (numerator) directly in transposed [d, (query,
    dir)] layout. Feature dim is chunked {128, 72} so the dominant
    stationary loads hit the fast-weight-load path (needs exactly 128
    stationary columns).
  - S and T use one PSUM bank per round each (round 0: mm/tp slots,
    round 1: S/T slots) so a round's vector post never reads a bank the
    next round's matmuls are writing (PE-write + DVE-read of one bank is
    a hardware collision). zh+S/T emission is software-pipelined one
    chunk late and the round post is spread over the next round's chunks.
  - s = T/S, with the reference's all-masked-row behavior reproduced
    exactly: S==0 => s = sum_m h[m,:]/128 (uniform softmax).
  - Fusion gate f, u, att_s and the final MLP run in bf16 (weights shipped
    as bf16 from the host, PE transposes in bf16); sigmoid is computed as
    0.5*tanh(0.5x)+0.5 to stay in the same activation-table set.

kernel(**inputs) takes the full unsharded inputs (as produced by
setup_inputs) and returns the full (8,) output; it shards batch across the
8 NeuronCores internally via run_bass_kernel_spmd.
"""

from contextlib import ExitStack

import numpy as np
import ml_dtypes

import concourse.bass as bass
import concourse.bacc as bacc
import concourse.tile as tile
from concourse import mybir

F32 = mybir.dt.float32
BF16 = mybir.dt.bfloat16
I32 = mybir.dt.int32
AF = mybir.ActivationFunctionType
ALU = mybir.AluOpType
AX = mybir.AxisListType

L = 128          # sequence length
D = 200          # feature dim
VOCAB = 32000
PAD = 1
N_CORES = 8
CHUNK_I = 16     # queries per G/z chunk
N_CHUNKS = L // CHUNK_I
C_VAL = 5.0
FQ = 2 * D       # 400 = one query pair worth of (i, d)

# feature-dim chunks: 128-col chunks get the PE fast-weight-load path
CH = ((0, 128), (128, 72))                       # D = 200
CH2 = ((0, 128), (128, 72), (200, 128), (328, 72))   # 2D = 400


def build_nc():
    nc = bacc.Bacc("TRN2", target_bir_lowering=False, debug=False)

    def din(name, shape, dt):
        return nc.dram_tensor(name, shape, dt, kind="ExternalInput").ap()

    x_idx_d = {"c": din("xc_idx", [L, 1], I32), "r": din("xr_idx", [L, 1], I32)}
    emb = din("emb", [VOCAB, D], F32)
    Wh = din("Wh", [D, D], F32)
    W1 = din("W1", [D, D], F32)
    W2 = din("W2", [D, D], F32)
    Wf1 = din("Wf1", [D, D], BF16)
    Wf2 = din("Wf2", [D, D], F32)
    Ws1 = din("Ws1", [2 * D, 2 * D], BF16)
    Ws = din("Ws", [2 * D, 2 * D], BF16)
    F1 = din("F1", [8 * D, D], BF16)
    F2 = din("F2", [D, 1], BF16)
    b_rep = din("b_rep", [L, D], F32)
    masks_d = {"c": din("masks_c", [L, 2 * L], BF16),
               "r": din("masks_r", [L, 2 * L], BF16)}
    ident_f = din("ident_f", [L, L], F32)
    ident_b = din("ident_b", [L, L], BF16)

    y_out = nc.dram_tensor("y", [1, 1], F32, kind="ExternalOutput").ap()

    with tile.TileContext(nc) as tc, ExitStack() as ctx:
        singles = ctx.enter_context(tc.tile_pool(name="singles", bufs=1))
        blockp = ctx.enter_context(tc.tile_pool(name="blockp", bufs=2))
        work = ctx.enter_context(tc.tile_pool(name="work", bufs=2))
        sml = ctx.enter_context(tc.tile_pool(name="sml", bufs=2))
        ps_hrep = ctx.enter_context(tc.tile_pool(name="ps_hrep", bufs=1, space="PSUM"))
        ps_st = ctx.enter_context(tc.tile_pool(name="ps_st", bufs=1, space="PSUM"))
        ps_mm = ctx.enter_context(tc.tile_pool(name="ps_mm", bufs=1, space="PSUM"))
        ps_tp = ctx.enter_context(tc.tile_pool(name="ps_tp", bufs=1, space="PSUM"))

        def _t(pool, shape, dt, tag, **kw):
            return pool.tile(shape, dt, name=tag, tag=tag, **kw)

        # DMA queue discipline: weights on sync, gathers/masks on gpsimd.
        # Nothing on scalar/vector — their compute queues are load-bearing.
        def load(ap_dram, shape, dt, tag=None, eng=None):
            t = _t(singles, shape, dt, tag)
            (eng or nc.sync).dma_start(out=t[:], in_=ap_dram)
            return t

        # ---- critical-path loads first: gathers + the h-chain weights ----
        gath = {}
        for blk in ("c", "r"):
            idx_sb = _t(sml, [L, 1], I32, "idx")
            nc.gpsimd.dma_start(out=idx_sb[:], in_=x_idx_d[blk])
            xemb = _t(sml, [L, D], F32, "xemb")
            nc.gpsimd.indirect_dma_start(
                out=xemb[:], out_offset=None, in_=emb,
                in_offset=bass.IndirectOffsetOnAxis(ap=idx_sb[:, :1], axis=0))
            gath[blk] = xemb

        identf_sb = load(ident_f, [L, L], F32, "idf")
        Wh_sb = [load(Wh[o:o + s, :], [s, D], F32, f"Wh{k}")
                 for k, (o, s) in enumerate(CH)]
        W1_sb = [load(W1[o:o + s, :], [s, D], F32, f"W1{k}")
                 for k, (o, s) in enumerate(CH)]
        W2_sb = [load(W2[o:o + s, :], [s, D], F32, f"W2{k}")
                 for k, (o, s) in enumerate(CH)]
        brep_sb = load(b_rep, [L, D], F32, "brep", eng=nc.gpsimd)
        mask_sb = {"c": load(masks_d["c"], [L, 2 * L], BF16, "mskc", eng=nc.gpsimd),
                   "r": load(masks_d["r"], [L, 2 * L], BF16, "mskr", eng=nc.gpsimd)}
        # later-use gate/head weights: tiles declared here, DMAs issued
        # after prep_c (sync queue is idle by then; keeps the critical
        # path DMAs in front)
        identb_sb = _t(singles, [L, L], BF16, "idb")
        Wf1_sb = [_t(singles, [s, D], BF16, f"Wg1{k}")
                  for k, (o, s) in enumerate(CH)]
        Wf2_sb = [_t(singles, [s, D], F32, f"Wg2{k}")
                  for k, (o, s) in enumerate(CH)]
        Ws1_sb = [_t(singles, [s, 2 * D], BF16, f"Ws1{k}")
                  for k, (o, s) in enumerate(CH2)]
        Ws_sb = [_t(singles, [s, 2 * D], BF16, f"Ws{k}")
                 for k, (o, s) in enumerate(CH2)]
        F1_sb = [_t(singles, [s, D], BF16, f"F1_{g}_{q}")
                 for g in range(4) for q, (o, s) in enumerate(CH2)]
        F2A_sb = _t(singles, [128, 1], BF16, "F2A")
        F2B_sb = _t(singles, [72, 1], BF16, "F2B")

        def load_late_weights():
            nc.sync.dma_start(out=identb_sb[:], in_=ident_b)
            for k, (o, s) in enumerate(CH):
                nc.sync.dma_start(out=Wf1_sb[k][:], in_=Wf1[o:o + s, :])
                nc.sync.dma_start(out=Wf2_sb[k][:], in_=Wf2[o:o + s, :])
            for q, (o, s) in enumerate(CH2):
                nc.sync.dma_start(out=Ws1_sb[q][:], in_=Ws1[o:o + s, :])
                nc.sync.dma_start(out=Ws_sb[q][:], in_=Ws[o:o + s, :])
            kc = 0
            for g in range(4):
                for q, (o, s) in enumerate(CH2):
                    nc.sync.dma_start(
                        out=F1_sb[kc][:],
                        in_=F1[g * 2 * D + o:g * 2 * D + o + s, :])
                    kc += 1
            nc.sync.dma_start(out=F2A_sb[:], in_=F2[0:128, :])
            nc.sync.dma_start(out=F2B_sb[:], in_=F2[128:200, :])

        ones2_bf = _t(singles, [2, L], BF16, "ones2bf")
        nc.vector.memset(ones2_bf[:], 1.0)

        cv_sb = {"c": _t(singles, [L, 4], F32, "cv"),
                 "r": _t(singles, [L, 4], F32, "rv")}

        TP_ONLY = ((ps_tp, "tp"),)
        TP_ROT = ((ps_tp, "tp"), (ps_st, "S"), (ps_st, "T"))

        def transpose_to(dst_ap, src_ap, n_par, n_free, slots=TP_ONLY, si=0,
                         dt=F32):
            pool, tag = slots[si % len(slots)]
            tp = _t(pool, [n_free, n_par], dt, tag, padded_shape=[n_free, 512])
            ident = identf_sb if dt == F32 else identb_sb
            nc.tensor.transpose(out=tp[:, :], in_=src_ap,
                                identity=ident[0:n_par, 0:n_par])
            nc.scalar.copy(dst_ap, tp[:, :])

        def transpose_ch(src_ap, n_par, n_free, tag, slots=TP_ONLY, si=0):
            dst = _t(work, [n_free, n_par], F32, tag)
            transpose_to(dst[:], src_ap, n_par, n_free, slots=slots, si=si)
            return dst

        def elu_from_psum(ps_ap, shape, tag, dt=F32):
            r = _t(work, shape, F32, "elur")
            nc.scalar.activation(r[:], ps_ap, AF.Relu)
            mn = _t(work, shape, F32, "elum")
            nc.vector.tensor_scalar_min(mn[:], ps_ap, 0.0)
            ex = _t(work, shape, F32, "elue")
            nc.scalar.activation(ex[:], mn[:], AF.Exp)
            o = _t(work, shape, dt, tag + "_o")
            nc.vector.scalar_tensor_tensor(o[:], r[:], -1.0, ex[:],
                                           op0=ALU.add, op1=ALU.add)
            return o

        def prep_block(blk):
            # ---------- h = elu(x @ Wh) (gather already issued) ----------
            xemb = gath[blk]

            # prep_c runs before any round work, so it may rotate its
            # transposes across the then-free tp/S psum slots; prep_r runs
            # mid-main where "S" belongs to round 1.
            pslots = (((ps_tp, "tp"), (ps_st, "S")) if blk == "c"
                      else TP_ONLY)
            veng = nc.vector
            xembT = [transpose_ch(xemb[:, o:o + s], L, s, f"xT{k}",
                                  slots=pslots, si=k)
                     for k, (o, s) in enumerate(CH)]
            hpre = _t(ps_hrep, [L, D], F32, "hrep0", padded_shape=[L, 1024])
            for k in range(2):
                nc.tensor.matmul(out=hpre[:], lhsT=xembT[k][:], rhs=Wh_sb[k][:],
                                 start=(k == 0), stop=(k == 1))
            h_sb = elu_from_psum(hpre[:], [L, D], "h")
            h_bf = _t(sml, [L, D], BF16, "hbf")
            veng.tensor_copy(h_bf[:], h_sb[:])

            hT = [transpose_ch(h_sb[:, o:o + s], L, s, f"hT{k}",
                               slots=pslots, si=k)
                  for k, (o, s) in enumerate(CH)]

            # ---------- h2 = h @ W2 and h1b = h @ W1 + b ----------
            h2ps = _t(ps_hrep, [L, D], F32, "hrep1", padded_shape=[L, 1024])
            for k in range(2):
                nc.tensor.matmul(out=h2ps[:], lhsT=hT[k][:], rhs=W2_sb[k][:],
                                 start=(k == 0), stop=(k == 1))
            h2_sb = _t(sml, [L, D], F32, "h2sb")
            nc.scalar.copy(h2_sb[:], h2ps[:])

            h1ps = _t(ps_hrep, [L, D], F32, "hrep0", padded_shape=[L, 1024])
            for k in range(2):
                nc.tensor.matmul(out=h1ps[:], lhsT=hT[k][:], rhs=W1_sb[k][:],
                                 start=(k == 0), stop=(k == 1))
            h1b = _t(sml, [L, D], F32, "h1b")
            nc.vector.tensor_add(h1b[:], h1ps[:], brep_sb[:])
            # exact bf16 hi/lo pair of h1+b, flattened to [2, 25600] via
            # sbuf->sbuf DMA (partition-major flatten)
            h1hi = _t(sml, [L, D], BF16, "h1hi")
            veng.tensor_copy(h1hi[:], h1b[:])
            h1rem = _t(sml, [L, D], F32, "h1rem")
            veng.tensor_sub(h1rem[:], h1b[:], h1hi[:])
            h1lo = _t(sml, [L, D], BF16, "h1lo")
            veng.tensor_copy(h1lo[:], h1rem[:])
            flathl = _t(blockp, [2, L * D], BF16, "flathl", bufs=1)
            nc.sync.dma_start(out=flathl[0:1, :], in_=h1hi[:])
            nc.gpsimd.dma_start(out=flathl[1:2, :], in_=h1lo[:])

            # HallT[:, ch] = sum_m h[m, d-chunk] as columns (fix rows)
            HallT = _t(sml, [L, 2], F32, "hallT")
            for k, (o, s) in enumerate(CH):
                nc.vector.tensor_reduce(out=HallT[0:s, k:k + 1], in_=hT[k][:],
                                        axis=AX.X, op=ALU.add)
            # dense h repeat via small sbuf->sbuf DMAs (engines stay free):
            # lets the zh multiply run dense bf16 at 2x instead of 1x.
            # slice-granular subtile deps let early chunks start after a
            # few slices.
            hrep16 = _t(sml, [L, CHUNK_I, D], BF16, "hrep16")
            for r16 in range(CHUNK_I):
                eng = nc.sync if r16 % 2 == 0 else nc.gpsimd
                eng.dma_start(out=hrep16[:, r16, :], in_=h_bf[:])
            return dict(h_sb=h_sb, h_bf=h_bf, hT=hT, h2_sb=h2_sb,
                        flathl=flathl, HallT=HallT, hrep16=hrep16)

        def main_block(blk, st_, mid=None):
            msk = mask_sb[blk]
            h_sb, h_bf, hT, h2_sb = (st_["h_sb"], st_["h_bf"], st_["hT"],
                                     st_["h2_sb"])
            flathl, HallT = st_["flathl"], st_["HallT"]
            hrep16 = st_["hrep16"]

            # ---------- main loop: G -> tanh -> exp -> zh -> S/T ----------
            # per round, S and T share ONE psum bank (S cols 0:256, T cols
            # 256:512); rounds use different banks so round r's vector post
            # never reads a bank round r+1's matmuls are writing (PE-write +
            # DVE-read of the same bank is a hardware collision).
            sT = {0: [_t(blockp, [s, L], BF16, f"sTf{c}")
                      for c, (o, s) in enumerate(CH)],
                  1: [_t(blockp, [s, L], BF16, f"sTb{c}")
                      for c, (o, s) in enumerate(CH)]}
            # ragged chunks: small leading chunks shorten the first-z latency
            CHUNKS = {0: ((0, 4), (4, 4), (8, 8), (16, 16), (32, 16),
                          (48, 8), (56, 4), (60, 4)),
                      1: ((64, 16), (80, 16), (96, 8), (104, 8), (112, 8),
                          (120, 4), (124, 4))}
            # software-pipelined emission: each chunk's zh+S/T matmuls are
            # emitted one chunk late (so the vector queue never stalls on
            # exp), and each round's post ops are spread over the next
            # round's chunks (disjoint psum banks make that overlap safe).
            pending = []
            pend_zh = []

            def emit_zh_st(z_sb, i0, cn, S_ps, T_ps):
                zh_sb = _t(work, [L, CHUNK_I, D], BF16, "zh", bufs=2)
                zr = z_sb[:].rearrange("p (a d) -> p a d", d=D)
                nv = cn - 4 if cn >= 16 else cn
                nc.vector.tensor_mul(
                    zh_sb[:, 0:nv, :], zr[:, 0:nv, :], hrep16[:, 0:nv, :])
                if nv < cn:
                    nc.gpsimd.tensor_mul(
                        zh_sb[:, nv:cn, :], zr[:, nv:cn, :],
                        hrep16[:, nv:cn, :])
                zh_2d = zh_sb[:].rearrange("p a d -> p (a d)")
                for iq in range(cn):
                    i = i0 + iq
                    j = i % 64
                    pm = msk[:, 2 * i:2 * i + 2]
                    for c, (o, s) in enumerate(CH):
                        co = 128 * c + 2 * j
                        nc.tensor.matmul(
                            out=S_ps[0:s, co:co + 2],
                            lhsT=z_sb[:, iq * D + o:iq * D + o + s],
                            rhs=pm, start=True, stop=True)
                        nc.tensor.matmul(
                            out=T_ps[0:s, co:co + 2],
                            lhsT=zh_2d[:, iq * D + o:iq * D + o + s],
                            rhs=pm, start=True, stop=True)

            def make_post(rnd, S_ps, T_ps):
                # ---- s = (T + ind*HallT) / (S + 128*ind), as closures ----
                # garbage rows 72:128 of the ch1 halves are never read.
                st = {}

                def p1():
                    st["ind"] = _t(work, [L, 2 * L], F32, "ind", bufs=1)
                    nc.vector.tensor_scalar(out=st["ind"][:], in0=S_ps[:, :],
                                            scalar1=0.0, scalar2=None,
                                            op0=ALU.is_equal)

                def p2():
                    S1 = _t(work, [L, 2 * L], F32, "S1", bufs=1)
                    nc.vector.scalar_tensor_tensor(S1[:], st["ind"][:], 128.0,
                                                   S_ps[:, :],
                                                   op0=ALU.mult, op1=ALU.add)
                    st["Sinv"] = _t(work, [L, 2 * L], F32, "Sinv", bufs=1)
                    nc.vector.reciprocal(st["Sinv"][:], S1[:])

                def p3():
                    TH = _t(work, [L, 2, L], F32, "TH", bufs=1)
                    nc.vector.tensor_mul(
                        TH[:], st["ind"][:].rearrange("p (a d) -> p a d", d=L),
                        HallT[:].unsqueeze(2).to_broadcast([L, 2, L]))
                    st["T1"] = _t(work, [L, 2 * L], F32, "T1", bufs=1)
                    nc.vector.tensor_add(st["T1"][:], T_ps[:, :],
                                         TH[:].rearrange("p a d -> p (a d)"))

                def p4(dire):
                    for c, (o, s) in enumerate(CH):
                        sl = slice(128 * c + dire, 128 * c + 128, 2)
                        nc.vector.tensor_mul(
                            sT[dire][c][:, 64 * rnd:64 * rnd + 64],
                            st["T1"][0:s, sl], st["Sinv"][0:s, sl])

                return [p1, p2, p3, lambda: p4(0), lambda: p4(1)]

            for rnd in range(2):
                # slot choice per (block, round): a block's round-0 slots
                # must not be the ones the PREVIOUS block's gates use
                # ("mm"/"tp"), or round 0 stalls until those gates drain.
                if (rnd == 0) == (blk == "c"):
                    S_ps = _t(ps_mm, [L, 2 * L], F32, "mm")
                    T_ps = _t(ps_tp, [L, 2 * L], F32, "tp")
                else:
                    S_ps = _t(ps_st, [L, 2 * L], F32, "S")
                    T_ps = _t(ps_st, [L, 2 * L], F32, "T")
                for (i0, cn) in CHUNKS[rnd]:
                    # h1+b broadcast to all partitions: k=2 hi/lo pair matmul
                    G_sb = _t(work, [L, CHUNK_I * D], F32, "G", bufs=2)
                    for hh in range(cn // 4):
                        hrep = _t(ps_hrep, [L, 2, 512], F32, f"hrep{hh % 2}")
                        for q in range(2):
                            o = (i0 + (2 * hh + q) * 2) * D
                            nc.tensor.matmul(out=hrep[:, q, 0:FQ],
                                             lhsT=ones2_bf[:],
                                             rhs=flathl[:, o:o + FQ],
                                             start=True, stop=True)
                        nc.vector.tensor_add(
                            G_sb[:, hh * 4 * D:(hh + 1) * 4 * D].rearrange(
                                "p (a b d) -> p a b d", b=2, d=D),
                            hrep[:, :, 0:FQ].rearrange(
                                "p a (b d) -> p a b d", d=D),
                            h2_sb[:].unsqueeze(1).unsqueeze(1).to_broadcast(
                                [L, 2, 2, D]))
                    A_sb = _t(work, [L, CHUNK_I * D], F32, "A", bufs=1)
                    nc.scalar.activation(A_sb[:, 0:cn * D], G_sb[:, 0:cn * D],
                                         AF.Tanh, scale=1.0 / C_VAL)
                    z_sb = _t(work, [L, CHUNK_I * D], BF16, "z", bufs=2)
                    nc.scalar.activation(z_sb[:, 0:cn * D], A_sb[:, 0:cn * D],
                                         AF.Exp, scale=C_VAL)
                    # flush: previous chunk's zh+S/T, then up to two spread
                    # post ops of the previous round
                    while pend_zh:
                        pend_zh.pop(0)()
                    for _ in range(2):
                        if pending:
                            pending.pop(0)()
                    pend_zh.append(
                        lambda z_sb=z_sb, i0=i0, cn=cn, S=S_ps, T=T_ps:
                        emit_zh_st(z_sb, i0, cn, S, T))
                # round done: queue the post; it must follow the last
                # chunk's zh+st, which pend_zh ordering guarantees
                pending.extend(make_post(rnd, S_ps, T_ps))
            # drain whatever is left (last chunk + round-1 post)
            while pend_zh:
                pend_zh.pop(0)()
            while pending:
                pending.pop(0)()

            # issue the next block's prep here: its engine work interleaves
            # with this block's gates in the queues (both independent)
            mid_ret = mid() if mid is not None else None

            # ---------- fusion gate f, u (in transposed space) ----------
            # block r's tail is the exposed end of the kernel: rotate its
            # transposes/matmuls across the then-idle psum slots.
            # block c's u-math is latency-tolerant (cv_c needed only at the
            # head): run it on gpsimd so block r's G-adds aren't queued
            # behind it on the DVE.
            rot = TP_ROT if blk == "r" else TP_ONLY
            ueng = nc.gpsimd if blk == "c" else nc.vector
            uT = {}
            for dire in range(2):
                if blk == "r" and dire == 1:
                    fps = _t(ps_st, [L, D], F32, "S", padded_shape=[L, 512])
                else:
                    fps = _t(ps_mm, [L, D], F32, "mm", padded_shape=[L, 512])
                for k in range(2):
                    nc.tensor.matmul(out=fps[:], lhsT=sT[dire][k][:],
                                     rhs=Wf1_sb[k][:],
                                     start=(k == 0), stop=False)
                for k in range(2):
                    nc.tensor.matmul(out=fps[:], lhsT=hT[k][:],
                                     rhs=Wf2_sb[k][:],
                                     start=False, stop=(k == 1))
                tsig = _t(work, [L, D], BF16, "tsig")
                nc.scalar.activation(tsig[:], fps[:], AF.Tanh, scale=0.5)
                for c, (o, s) in enumerate(CH):
                    fT = _t(work, [s, L], F32, f"fT{dire}{c}")
                    transpose_to(fT[:], tsig[:, o:o + s], L, s,
                                 slots=rot, si=2 * dire + c, dt=BF16)
                    ueng.tensor_scalar(out=fT[:], in0=fT[:], scalar1=0.5,
                                        scalar2=0.5, op0=ALU.mult,
                                        op1=ALU.add)
                    # uT = sT + fT * (hT - sT)
                    dt_ = _t(work, [s, L], F32, f"d{dire}{c}")
                    ueng.tensor_sub(dt_[:], hT[c][:], sT[dire][c][:])
                    ueng.tensor_mul(dt_[:], fT[:], dt_[:])
                    u = _t(blockp, [s, L], BF16, f"uT{dire}{c}")
                    ueng.tensor_add(u[:], sT[dire][c][:], dt_[:])
                    uT[(dire, c)] = u
            uT_list = [uT[(0, 0)], uT[(0, 1)], uT[(1, 0)], uT[(1, 1)]]

            # ---------- att_s = elu(u @ Ws1) @ Ws ; cv = sum_i u*att_s ----------
            wps = _t(ps_mm, [L, 2 * D], F32, "mm", padded_shape=[L, 512])
            for q in range(4):
                nc.tensor.matmul(out=wps[:], lhsT=uT_list[q][:], rhs=Ws1_sb[q][:],
                                 start=(q == 0), stop=(q == 3))
            w_sb = elu_from_psum(wps[:], [L, 2 * D], "w", dt=BF16)
            wT = []
            for q, (o, s) in enumerate(CH2):
                dst = _t(work, [s, L], BF16, f"wT{q}")
                transpose_to(dst[:], w_sb[:, o:o + s], L, s,
                             slots=rot, si=q, dt=BF16)
                wT.append(dst)
            aps = _t(ps_mm, [L, 2 * D], F32, "mm", padded_shape=[L, 512])
            for q in range(4):
                nc.tensor.matmul(out=aps[:], lhsT=wT[q][:], rhs=Ws_sb[q][:],
                                 start=(q == 0), stop=(q == 3))
            atts_sb = _t(work, [L, 2 * D], BF16, "atts")
            nc.scalar.copy(atts_sb[:], aps[:])
            for q, (o, s) in enumerate(CH2):
                pool, tag = rot[q % len(rot)]
                aT = _t(pool, [s, L], BF16, tag, padded_shape=[s, 512])
                nc.tensor.transpose(out=aT[:, :],
                                    in_=atts_sb[:, o:o + s],
                                    identity=identb_sb[:, :])
                vT = _t(work, [s, L], F32, "vT")
                nc.vector.scalar_tensor_tensor(
                    vT[:], uT_list[q][:], 1.0, aT[:, :],
                    op0=ALU.mult, op1=ALU.mult,
                    accum_out=cv_sb[blk][0:s, q:q + 1])
            return mid_ret

        st_c = prep_block("c")
        load_late_weights()
        st_r = main_block("c", st_c, mid=lambda: prep_block("r"))
        main_block("r", st_r)

        # ---------- head: feat = [cv, rv, cv-rv, cv*rv]; y ----------
        diff = _t(singles, [L, 4], F32, "diff")
        nc.vector.tensor_sub(diff[:], cv_sb["c"][:], cv_sb["r"][:])
        prod = _t(singles, [L, 4], F32, "prod")
        nc.vector.tensor_mul(prod[:], cv_sb["c"][:], cv_sb["r"][:])
        groups_f = [cv_sb["c"], cv_sb["r"], diff, prod]
        groups = []
        for gi, g in enumerate(groups_f):
            gb = _t(singles, [L, 4], BF16, f"gb{gi}")
            nc.vector.tensor_copy(gb[:], g[:])
            groups.append(gb)

        y1A = _t(ps_st, [128, 1], F32, "S", padded_shape=[128, 512])
        y1B = _t(ps_st, [72, 1], F32, "T", padded_shape=[72, 512])
        nmm = 4 * len(CH2)
        kc = 0
        for g in range(4):
            for q, (o, s) in enumerate(CH2):
                col = groups[g][0:s, q:q + 1]
                nc.tensor.matmul(out=y1A[:], lhsT=F1_sb[kc][:, 0:128], rhs=col,
                                 start=(kc == 0), stop=(kc == nmm - 1))
                kc += 1
        kc = 0
        for g in range(4):
            for q, (o, s) in enumerate(CH2):
                col = groups[g][0:s, q:q + 1]
                nc.tensor.matmul(out=y1B[:], lhsT=F1_sb[kc][:, 128:200], rhs=col,
                                 start=(kc == 0), stop=(kc == nmm - 1))
                kc += 1
        r1A = _t(sml, [128, 1], BF16, "r1A")
        nc.scalar.activation(r1A[:], y1A[:], AF.Relu)
        r1B = _t(sml, [72, 1], BF16, "r1B")
        nc.scalar.activation(r1B[:], y1B[:], AF.Relu)
        yps = _t(ps_mm, [L, 2 * D], F32, "mm", padded_shape=[L, 512])[0:1, 0:1]
        nc.tensor.matmul(out=yps[:], lhsT=r1A[:], rhs=F2A_sb[:],
                         start=True, stop=False)
        nc.tensor.matmul(out=yps[:], lhsT=r1B[:], rhs=F2B_sb[:],
                         start=False, stop=True)
        y_sb = _t(sml, [1, 1], F32, "ysb")
        nc.scalar.copy(y_sb[:], yps[:])
        nc.sync.dma_start(out=y_out, in_=y_sb[:])

    nc.compile()
    return nc


def _build_masks(ids):
    """[128, 256] bf16: col 2i+0 = fw col for query i (keys m>i), 2i+1 = bw
    (m<i); pad keys and pad queries zero the column."""
    np1 = (ids != PAD).astype(np.float32)
    m = np.arange(L)
    fw = (m[:, None] > m[None, :]).astype(np.float32) * np1[:, None] * np1[None, :]
    bw = (m[:, None] < m[None, :]).astype(np.float32) * np1[:, None] * np1[None, :]
    out = np.empty((L, 2 * L), np.float32)
    out[:, 0::2] = fw
    out[:, 1::2] = bw
    return out.astype(ml_dtypes.bfloat16)


def make_in_maps(inputs):
    x1 = np.asarray(inputs["x1"]).astype(np.int64)
    x2 = np.asarray(inputs["x2"]).astype(np.int64)
    f32 = lambda k: np.ascontiguousarray(np.asarray(inputs[k], np.float32))
    bf16 = lambda k: np.ascontiguousarray(
        np.asarray(inputs[k], np.float32).astype(ml_dtypes.bfloat16))
    emb = f32("emb_w")
    shared = {
        "emb": emb,
        "Wh": f32("Wh_w"), "W1": f32("W1_w"), "W2": f32("W2_w"),
        "Wf1": bf16("Wf1_w"), "Wf2": f32("Wf2_w"),
        "Ws1": bf16("Ws1_w"), "Ws": bf16("Ws_w"),
        "F1": bf16("F1_w"), "F2": bf16("F2_w").reshape(D, 1),
        "b_rep": np.tile(f32("b").reshape(1, D), (L, 1)),
        "ident_f": np.eye(L, dtype=np.float32),
        "ident_b": np.eye(L, dtype=np.float32).astype(ml_dtypes.bfloat16),
    }
    in_maps = []
    for bidx in range(N_CORES):
        m = dict(shared)
        m["xc_idx"] = x1[bidx].reshape(L, 1).astype(np.int32)
        m["xr_idx"] = x2[bidx].reshape(L, 1).astype(np.int32)
        m["masks_c"] = _build_masks(x1[bidx])
        m["masks_r"] = _build_masks(x2[bidx])
        in_maps.append(m)
    return in_maps


_NC_CACHE = {}


def get_nc():
    if "nc" not in _NC_CACHE:
        _NC_CACHE["nc"] = build_nc()
    return _NC_CACHE["nc"]


def kernel(**inputs) -> np.ndarray:
    from concourse.bass_utils import run_bass_kernel_spmd
    nc = get_nc()
    in_maps = make_in_maps(inputs)
    res = run_bass_kernel_spmd(nc, in_maps, list(range(N_CORES)))
    y = np.array([np.asarray(res.results[i]["y"]).reshape(-1)[0]
                  for i in range(N_CORES)], dtype=np.float32)
    return y


# revision 60
# speedup vs baseline: 1.0114x; 1.0114x over previous
"""DiSAN Trainium2 Bass kernel — 8-core data parallel (one example per core).

Per-core layout (one batch example, both text blocks x1/x2):
  - m (key token, 128) on SBUF partitions; (i=query, d=feature) on free axis.
  - att pre-activation G[m, i, d] = h1[i,d]+b[d] + h2[m,d] is built on the
    TensorEngine: rank-1 broadcast of (h1+b) with a ones lhsT as an exact
    bf16 hi/lo pair accumulated in PSUM; h2 added by the Vector/GpSimd
    engines (alternating, to keep the DVE off the critical path).
  - ScalarEngine: A = tanh(G/c) fp32, z = exp(c*A) bf16 (Tanh/Exp share one
    activation-table set — no table switches). 16-query chunks amortize the
    per-instruction overhead.
  - TensorEngine: per-query "flipped" matmuls — lhsT = z (or z*h) slice
    [m=128, d-chunk], rhs = the query's fw/bw 0/1 bf16 mask columns
    [m=128, 2] — contract over m, producing the directional softmax sums S
    (denominator) and T (numerator) directly in transposed [d, (query,
    dir)] layout. Feature dim is chunked {128, 72} so the dominant
    stationary loads hit the fast-weight-load path (needs exactly 128
    stationary columns).
  - S and T use one PSUM bank per round each (round 0: mm/tp slots,
    round 1: S/T slots) so a round's vector post never reads a bank the
    next round's matmuls are writing (PE-write + DVE-read of one bank is
    a hardware collision). zh+S/T emission is software-pipelined one
    chunk late and the round post is spread over the next round's chunks.
  - s = T/S, with the reference's all-masked-row behavior reproduced
    exactly: S==0 => s = sum_m h[m,:]/128 (uniform softmax).
  - Fusion gate f, u, att_s and the final MLP run in bf16 (weights shipped
    as bf16 from the host, PE transposes in bf16); sigmoid is computed as
    0.5*tanh(0.5x)+0.5 to stay in the same activation-table set.

kernel(**inputs) takes the full unsharded inputs (as produced by
setup_inputs) and returns the full (8,) output; it shards batch across the
8 NeuronCores internally via run_bass_kernel_spmd.
"""

from contextlib import ExitStack

import numpy as np
import ml_dtypes

import concourse.bass as bass
import concourse.bacc as bacc
import concourse.tile as tile
from concourse import mybir

F32 = mybir.dt.float32
BF16 = mybir.dt.bfloat16
I32 = mybir.dt.int32
AF = mybir.ActivationFunctionType
ALU = mybir.AluOpType
AX = mybir.AxisListType

L = 128          # sequence length
D = 200          # feature dim
VOCAB = 32000
PAD = 1
N_CORES = 8
CHUNK_I = 16     # queries per G/z chunk
N_CHUNKS = L // CHUNK_I
C_VAL = 5.0
FQ = 2 * D       # 400 = one query pair worth of (i, d)

# feature-dim chunks: 128-col chunks get the PE fast-weight-load path
CH = ((0, 128), (128, 72))                       # D = 200
CH2 = ((0, 128), (128, 72), (200, 128), (328, 72))   # 2D = 400


def build_nc():
    nc = bacc.Bacc("TRN2", target_bir_lowering=False, debug=False)

    def din(name, shape, dt):
        return nc.dram_tensor(name, shape, dt, kind="ExternalInput").ap()

    x_idx_d = {"c": din("xc_idx", [L, 1], I32), "r": din("xr_idx", [L, 1], I32)}
    emb = din("emb", [VOCAB, D], F32)
    Wh = din("Wh", [D, D], F32)
    W1 = din("W1", [D, D], F32)
    W2 = din("W2", [D, D], F32)
    Wf1 = din("Wf1", [D, D], BF16)
    Wf2 = din("Wf2", [D, D], F32)
    Ws1 = din("Ws1", [2 * D, 2 * D], BF16)
    Ws = din("Ws", [2 * D, 2 * D], BF16)
    F1 = din("F1", [8 * D, D], BF16)
    F2 = din("F2", [D, 1], BF16)
    b_rep = din("b_rep", [L, D], F32)
    masks_d = {"c": din("masks_c", [L, 2 * L], BF16),
               "r": din("masks_r", [L, 2 * L], BF16)}
    ident_f = din("ident_f", [L, L], F32)
    ident_b = din("ident_b", [L, L], BF16)

    y_out = nc.dram_tensor("y", [1, 1], F32, kind="ExternalOutput").ap()

    with tile.TileContext(nc) as tc, ExitStack() as ctx:
        singles = ctx.enter_context(tc.tile_pool(name="singles", bufs=1))
        blockp = ctx.enter_context(tc.tile_pool(name="blockp", bufs=2))
        work = ctx.enter_context(tc.tile_pool(name="work", bufs=2))
        sml = ctx.enter_context(tc.tile_pool(name="sml", bufs=2))
        ps_hrep = ctx.enter_context(tc.tile_pool(name="ps_hrep", bufs=1, space="PSUM"))
        ps_st = ctx.enter_context(tc.tile_pool(name="ps_st", bufs=1, space="PSUM"))
        ps_mm = ctx.enter_context(tc.tile_pool(name="ps_mm", bufs=1, space="PSUM"))
        ps_tp = ctx.enter_context(tc.tile_pool(name="ps_tp", bufs=1, space="PSUM"))

        def _t(pool, shape, dt, tag, **kw):
            return pool.tile(shape, dt, name=tag, tag=tag, **kw)

        # DMA queue discipline: weights on sync, gathers/masks on gpsimd.
        # Nothing on scalar/vector — their compute queues are load-bearing.
        def load(ap_dram, shape, dt, tag=None, eng=None):
            t = _t(singles, shape, dt, tag)
            (eng or nc.sync).dma_start(out=t[:], in_=ap_dram)
            return t

        # ---- critical-path loads first: gathers + the h-chain weights ----
        gath = {}
        for blk in ("c", "r"):
            idx_sb = _t(sml, [L, 1], I32, "idx")
            nc.gpsimd.dma_start(out=idx_sb[:], in_=x_idx_d[blk])
            xemb = _t(sml, [L, D], F32, "xemb")
            nc.gpsimd.indirect_dma_start(
                out=xemb[:], out_offset=None, in_=emb,
                in_offset=bass.IndirectOffsetOnAxis(ap=idx_sb[:, :1], axis=0))
            gath[blk] = xemb

        identf_sb = load(ident_f, [L, L], F32, "idf")
        Wh_sb = [load(Wh[o:o + s, :], [s, D], F32, f"Wh{k}")
                 for k, (o, s) in enumerate(CH)]
        W1_sb = [load(W1[o:o + s, :], [s, D], F32, f"W1{k}")
                 for k, (o, s) in enumerate(CH)]
        W2_sb = [load(W2[o:o + s, :], [s, D], F32, f"W2{k}")
                 for k, (o, s) in enumerate(CH)]
        brep_sb = load(b_rep, [L, D], F32, "brep", eng=nc.gpsimd)
        mask_sb = {"c": load(masks_d["c"], [L, 2 * L], BF16, "mskc", eng=nc.gpsimd),
                   "r": load(masks_d["r"], [L, 2 * L], BF16, "mskr", eng=nc.gpsimd)}
        # later-use gate/head weights: tiles declared here, DMAs issued
        # after prep_c (sync queue is idle by then; keeps the critical
        # path DMAs in front)
        identb_sb = _t(singles, [L, L], BF16, "idb")
        Wf1_sb = [_t(singles, [s, D], BF16, f"Wg1{k}")
                  for k, (o, s) in enumerate(CH)]
        Wf2_sb = [_t(singles, [s, D], F32, f"Wg2{k}")
                  for k, (o, s) in enumerate(CH)]
        Ws1_sb = [_t(singles, [s, 2 * D], BF16, f"Ws1{k}")
                  for k, (o, s) in enumerate(CH2)]
        Ws_sb = [_t(singles, [s, 2 * D], BF16, f"Ws{k}")
                 for k, (o, s) in enumerate(CH2)]
        F1_sb = [_t(singles, [s, D], BF16, f"F1_{g}_{q}")
                 for g in range(4) for q, (o, s) in enumerate(CH2)]
        F2A_sb = _t(singles, [128, 1], BF16, "F2A")
        F2B_sb = _t(singles, [72, 1], BF16, "F2B")

        def load_late_weights():
            nc.sync.dma_start(out=identb_sb[:], in_=ident_b)
            for k, (o, s) in enumerate(CH):
                nc.sync.dma_start(out=Wf1_sb[k][:], in_=Wf1[o:o + s, :])
                nc.sync.dma_start(out=Wf2_sb[k][:], in_=Wf2[o:o + s, :])
            for q, (o, s) in enumerate(CH2):
                nc.sync.dma_start(out=Ws1_sb[q][:], in_=Ws1[o:o + s, :])
                nc.sync.dma_start(out=Ws_sb[q][:], in_=Ws[o:o + s, :])
            kc = 0
            for g in range(4):
                for q, (o, s) in enumerate(CH2):
                    nc.sync.dma_start(
                        out=F1_sb[kc][:],
                        in_=F1[g * 2 * D + o:g * 2 * D + o + s, :])
                    kc += 1
            nc.sync.dma_start(out=F2A_sb[:], in_=F2[0:128, :])
            nc.sync.dma_start(out=F2B_sb[:], in_=F2[128:200, :])

        ones2_bf = _t(singles, [2, L], BF16, "ones2bf")
        nc.vector.memset(ones2_bf[:], 1.0)

        cv_sb = {"c": _t(singles, [L, 4], F32, "cv"),
                 "r": _t(singles, [L, 4], F32, "rv")}

        TP_ONLY = ((ps_tp, "tp"),)
        TP_ROT = ((ps_tp, "tp"), (ps_st, "S"), (ps_st, "T"))

        def transpose_to(dst_ap, src_ap, n_par, n_free, slots=TP_ONLY, si=0,
                         dt=F32):
            pool, tag = slots[si % len(slots)]
            tp = _t(pool, [n_free, n_par], dt, tag, padded_shape=[n_free, 512])
            ident = identf_sb if dt == F32 else identb_sb
            nc.tensor.transpose(out=tp[:, :], in_=src_ap,
                                identity=ident[0:n_par, 0:n_par])
            nc.scalar.copy(dst_ap, tp[:, :])

        def transpose_ch(src_ap, n_par, n_free, tag, slots=TP_ONLY, si=0):
            dst = _t(work, [n_free, n_par], F32, tag)
            transpose_to(dst[:], src_ap, n_par, n_free, slots=slots, si=si)
            return dst

        def elu_from_psum(ps_ap, shape, tag, dt=F32):
            r = _t(work, shape, F32, "elur")
            nc.scalar.activation(r[:], ps_ap, AF.Relu)
            mn = _t(work, shape, F32, "elum")
            nc.vector.tensor_scalar_min(mn[:], ps_ap, 0.0)
            ex = _t(work, shape, F32, "elue")
            nc.scalar.activation(ex[:], mn[:], AF.Exp)
            o = _t(work, shape, dt, tag + "_o")
            nc.vector.scalar_tensor_tensor(o[:], r[:], -1.0, ex[:],
                                           op0=ALU.add, op1=ALU.add)
            return o

        def prep_block(blk):
            # ---------- h = elu(x @ Wh) (gather already issued) ----------
            xemb = gath[blk]

            # prep_c runs before any round work, so it may rotate its
            # transposes across the then-free tp/S psum slots; prep_r runs
            # mid-main where "S" belongs to round 1.
            pslots = (((ps_tp, "tp"), (ps_st, "S")) if blk == "c"
                      else TP_ONLY)
            veng = nc.vector
            xembT = [transpose_ch(xemb[:, o:o + s], L, s, f"xT{k}",
                                  slots=pslots, si=k)
                     for k, (o, s) in enumerate(CH)]
            hpre = _t(ps_hrep, [L, D], F32, "hrep0", padded_shape=[L, 1024])
            for k in range(2):
                nc.tensor.matmul(out=hpre[:], lhsT=xembT[k][:], rhs=Wh_sb[k][:],
                                 start=(k == 0), stop=(k == 1))
            h_sb = elu_from_psum(hpre[:], [L, D], "h")
            h_bf = _t(sml, [L, D], BF16, "hbf")
            veng.tensor_copy(h_bf[:], h_sb[:])

            hT = [transpose_ch(h_sb[:, o:o + s], L, s, f"hT{k}",
                               slots=pslots, si=k)
                  for k, (o, s) in enumerate(CH)]

            # ---------- h2 = h @ W2 and h1b = h @ W1 + b ----------
            h2ps = _t(ps_hrep, [L, D], F32, "hrep1", padded_shape=[L, 1024])
            for k in range(2):
                nc.tensor.matmul(out=h2ps[:], lhsT=hT[k][:], rhs=W2_sb[k][:],
                                 start=(k == 0), stop=(k == 1))
            h2_sb = _t(sml, [L, D], F32, "h2sb")
            nc.scalar.copy(h2_sb[:], h2ps[:])

            h1ps = _t(ps_hrep, [L, D], F32, "hrep0", padded_shape=[L, 1024])
            for k in range(2):
                nc.tensor.matmul(out=h1ps[:], lhsT=hT[k][:], rhs=W1_sb[k][:],
                                 start=(k == 0), stop=(k == 1))
            h1b = _t(sml, [L, D], F32, "h1b")
            nc.vector.tensor_add(h1b[:], h1ps[:], brep_sb[:])
            # exact bf16 hi/lo pair of h1+b, flattened to [2, 25600] via
            # sbuf->sbuf DMA (partition-major flatten)
            h1hi = _t(sml, [L, D], BF16, "h1hi")
            veng.tensor_copy(h1hi[:], h1b[:])
            h1rem = _t(sml, [L, D], F32, "h1rem")
            veng.tensor_sub(h1rem[:], h1b[:], h1hi[:])
            h1lo = _t(sml, [L, D], BF16, "h1lo")
            veng.tensor_copy(h1lo[:], h1rem[:])
            flathl = _t(blockp, [2, L * D], BF16, "flathl", bufs=1)
            nc.sync.dma_start(out=flathl[0:1, :], in_=h1hi[:])
            nc.gpsimd.dma_start(out=flathl[1:2, :], in_=h1lo[:])

            # HallT[:, ch] = sum_m h[m, d-chunk] as columns (fix rows)
            HallT = _t(sml, [L, 2], F32, "hallT")
            for k, (o, s) in enumerate(CH):
                nc.vector.tensor_reduce(out=HallT[0:s, k:k + 1], in_=hT[k][:],
                                        axis=AX.X, op=ALU.add)
            # dense h repeat via small sbuf->sbuf DMAs (engines stay free):
            # lets the zh multiply run dense bf16 at 2x instead of 1x.
            # slice-granular subtile deps let early chunks start after a
            # few slices.
            hrep16 = _t(sml, [L, CHUNK_I, D], BF16, "hrep16")
            for r16 in range(CHUNK_I):
                eng = nc.sync if r16 % 2 == 0 else nc.gpsimd
                eng.dma_start(out=hrep16[:, r16, :], in_=h_bf[:])
            return dict(h_sb=h_sb, h_bf=h_bf, hT=hT, h2_sb=h2_sb,
                        flathl=flathl, HallT=HallT, hrep16=hrep16)

        def main_block(blk, st_, mid=None):
            msk = mask_sb[blk]
            h_sb, h_bf, hT, h2_sb = (st_["h_sb"], st_["h_bf"], st_["hT"],
                                     st_["h2_sb"])
            flathl, HallT = st_["flathl"], st_["HallT"]
            hrep16 = st_["hrep16"]

            # ---------- main loop: G -> tanh -> exp -> zh -> S/T ----------
            # per round, S and T share ONE psum bank (S cols 0:256, T cols
            # 256:512); rounds use different banks so round r's vector post
            # never reads a bank round r+1's matmuls are writing (PE-write +
            # DVE-read of the same bank is a hardware collision).
            sT = {0: [_t(blockp, [s, L], BF16, f"sTf{c}")
                      for c, (o, s) in enumerate(CH)],
                  1: [_t(blockp, [s, L], BF16, f"sTb{c}")
                      for c, (o, s) in enumerate(CH)]}
            # ragged chunks: small leading chunks shorten the first-z latency
            CHUNKS = {0: ((0, 4), (4, 4), (8, 8), (16, 16), (32, 16), (48, 16)),
                      1: ((64, 16), (80, 8), (88, 8), (96, 8), (104, 8),
                          (112, 8), (120, 4), (124, 4))}
            # software-pipelined emission: each chunk's zh+S/T matmuls are
            # emitted one chunk late (so the vector queue never stalls on
            # exp), and each round's post ops are spread over the next
            # round's chunks (disjoint psum banks make that overlap safe).
            pending = []
            pend_zh = []

            def emit_zh_st(z_sb, i0, cn, S_ps, T_ps):
                zh_sb = _t(work, [L, CHUNK_I, D], BF16, "zh", bufs=2)
                zr = z_sb[:].rearrange("p (a d) -> p a d", d=D)
                nv = cn - 4 if cn >= 16 else cn
                nc.vector.tensor_mul(
                    zh_sb[:, 0:nv, :], zr[:, 0:nv, :], hrep16[:, 0:nv, :])
                if nv < cn:
                    nc.gpsimd.tensor_mul(
                        zh_sb[:, nv:cn, :], zr[:, nv:cn, :],
                        hrep16[:, nv:cn, :])
                zh_2d = zh_sb[:].rearrange("p a d -> p (a d)")
                for iq in range(cn):
                    i = i0 + iq
                    j = i % 64
                    pm = msk[:, 2 * i:2 * i + 2]
                    for c, (o, s) in enumerate(CH):
                        co = 128 * c + 2 * j
                        nc.tensor.matmul(
                            out=S_ps[0:s, co:co + 2],
                            lhsT=z_sb[:, iq * D + o:iq * D + o + s],
                            rhs=pm, start=True, stop=True)
                        nc.tensor.matmul(
                            out=T_ps[0:s, co:co + 2],
                            lhsT=zh_2d[:, iq * D + o:iq * D + o + s],
                            rhs=pm, start=True, stop=True)

            def make_post(rnd, S_ps, T_ps):
                # ---- s = (T + ind*HallT) / (S + 128*ind), as closures ----
                # garbage rows 72:128 of the ch1 halves are never read.
                st = {}

                def p1():
                    st["ind"] = _t(work, [L, 2 * L], F32, "ind", bufs=1)
                    nc.vector.tensor_scalar(out=st["ind"][:], in0=S_ps[:, :],
                                            scalar1=0.0, scalar2=None,
                                            op0=ALU.is_equal)

                def p2():
                    S1 = _t(work, [L, 2 * L], F32, "S1", bufs=1)
                    nc.vector.scalar_tensor_tensor(S1[:], st["ind"][:], 128.0,
                                                   S_ps[:, :],
                                                   op0=ALU.mult, op1=ALU.add)
                    st["Sinv"] = _t(work, [L, 2 * L], F32, "Sinv", bufs=1)
                    nc.vector.reciprocal(st["Sinv"][:], S1[:])

                def p3():
                    TH = _t(work, [L, 2, L], F32, "TH", bufs=1)
                    nc.vector.tensor_mul(
                        TH[:], st["ind"][:].rearrange("p (a d) -> p a d", d=L),
                        HallT[:].unsqueeze(2).to_broadcast([L, 2, L]))
                    st["T1"] = _t(work, [L, 2 * L], F32, "T1", bufs=1)
                    nc.vector.tensor_add(st["T1"][:], T_ps[:, :],
                                         TH[:].rearrange("p a d -> p (a d)"))

                def p4(dire):
                    for c, (o, s) in enumerate(CH):
                        sl = slice(128 * c + dire, 128 * c + 128, 2)
                        nc.vector.tensor_mul(
                            sT[dire][c][:, 64 * rnd:64 * rnd + 64],
                            st["T1"][0:s, sl], st["Sinv"][0:s, sl])

                return [p1, p2, p3, lambda: p4(0), lambda: p4(1)]

            for rnd in range(2):
                # slot choice per (block, round): a block's round-0 slots
                # must not be the ones the PREVIOUS block's gates use
                # ("mm"/"tp"), or round 0 stalls until those gates drain.
                if (rnd == 0) == (blk == "c"):
                    S_ps = _t(ps_mm, [L, 2 * L], F32, "mm")
                    T_ps = _t(ps_tp, [L, 2 * L], F32, "tp")
                else:
                    S_ps = _t(ps_st, [L, 2 * L], F32, "S")
                    T_ps = _t(ps_st, [L, 2 * L], F32, "T")
                for (i0, cn) in CHUNKS[rnd]:
                    # h1+b broadcast to all partitions: k=2 hi/lo pair matmul
                    G_sb = _t(work, [L, CHUNK_I * D], F32, "G", bufs=2)
                    for hh in range(cn // 4):
                        hrep = _t(ps_hrep, [L, 2, 512], F32, f"hrep{hh % 2}")
                        for q in range(2):
                            o = (i0 + (2 * hh + q) * 2) * D
                            nc.tensor.matmul(out=hrep[:, q, 0:FQ],
                                             lhsT=ones2_bf[:],
                                             rhs=flathl[:, o:o + FQ],
                                             start=True, stop=True)
                        nc.vector.tensor_add(
                            G_sb[:, hh * 4 * D:(hh + 1) * 4 * D].rearrange(
                                "p (a b d) -> p a b d", b=2, d=D),
                            hrep[:, :, 0:FQ].rearrange(
                                "p a (b d) -> p a b d", d=D),
                            h2_sb[:].unsqueeze(1).unsqueeze(1).to_broadcast(
                                [L, 2, 2, D]))
                    A_sb = _t(work, [L, CHUNK_I * D], F32, "A", bufs=1)
                    nc.scalar.activation(A_sb[:, 0:cn * D], G_sb[:, 0:cn * D],
                                         AF.Tanh, scale=1.0 / C_VAL)
                    z_sb = _t(work, [L, CHUNK_I * D], BF16, "z", bufs=2)
                    nc.scalar.activation(z_sb[:, 0:cn * D], A_sb[:, 0:cn * D],
                                         AF.Exp, scale=C_VAL)
                    # flush: previous chunk's zh+S/T, then up to two spread
                    # post ops of the previous round
                    while pend_zh:
                        pend_zh.pop(0)()
                    for _ in range(2):
                        if pending:
                            pending.pop(0)()
                    pend_zh.append(
                        lambda z_sb=z_sb, i0=i0, cn=cn, S=S_ps, T=T_ps:
                        emit_zh_st(z_sb, i0, cn, S, T))
                # round done: queue the post; it must follow the last
                # chunk's zh+st, which pend_zh ordering guarantees
                pending.extend(make_post(rnd, S_ps, T_ps))
            # drain whatever is left (last chunk + round-1 post)
            while pend_zh:
                pend_zh.pop(0)()
            while pending:
                pending.pop(0)()

            # issue the next block's prep here: its engine work interleaves
            # with this block's gates in the queues (both independent)
            mid_ret = mid() if mid is not None else None

            # ---------- fusion gate f, u (in transposed space) ----------
            # block r's tail is the exposed end of the kernel: rotate its
            # transposes/matmuls across the then-idle psum slots.
            # block c's u-math is latency-tolerant (cv_c needed only at the
            # head): run it on gpsimd so block r's G-adds aren't queued
            # behind it on the DVE.
            rot = TP_ROT if blk == "r" else TP_ONLY
            ueng = nc.gpsimd if blk == "c" else nc.vector
            uT = {}
            for dire in range(2):
                if blk == "r" and dire == 1:
                    fps = _t(ps_st, [L, D], F32, "S", padded_shape=[L, 512])
                else:
                    fps = _t(ps_mm, [L, D], F32, "mm", padded_shape=[L, 512])
                for k in range(2):
                    nc.tensor.matmul(out=fps[:], lhsT=sT[dire][k][:],
                                     rhs=Wf1_sb[k][:],
                                     start=(k == 0), stop=False)
                for k in range(2):
                    nc.tensor.matmul(out=fps[:], lhsT=hT[k][:],
                                     rhs=Wf2_sb[k][:],
                                     start=False, stop=(k == 1))
                tsig = _t(work, [L, D], BF16, "tsig")
                nc.scalar.activation(tsig[:], fps[:], AF.Tanh, scale=0.5)
                for c, (o, s) in enumerate(CH):
                    fT = _t(work, [s, L], F32, f"fT{dire}{c}")
                    transpose_to(fT[:], tsig[:, o:o + s], L, s,
                                 slots=rot, si=2 * dire + c, dt=BF16)
                    ueng.tensor_scalar(out=fT[:], in0=fT[:], scalar1=0.5,
                                        scalar2=0.5, op0=ALU.mult,
                                        op1=ALU.add)
                    # uT = sT + fT * (hT - sT)
                    dt_ = _t(work, [s, L], F32, f"d{dire}{c}")
                    ueng.tensor_sub(dt_[:], hT[c][:], sT[dire][c][:])
                    ueng.tensor_mul(dt_[:], fT[:], dt_[:])
                    u = _t(blockp, [s, L], BF16, f"uT{dire}{c}")
                    ueng.tensor_add(u[:], sT[dire][c][:], dt_[:])
                    uT[(dire, c)] = u
            uT_list = [uT[(0, 0)], uT[(0, 1)], uT[(1, 0)], uT[(1, 1)]]

            # ---------- att_s = elu(u @ Ws1) @ Ws ; cv = sum_i u*att_s ----------
            wps = _t(ps_mm, [L, 2 * D], F32, "mm", padded_shape=[L, 512])
            for q in range(4):
                nc.tensor.matmul(out=wps[:], lhsT=uT_list[q][:], rhs=Ws1_sb[q][:],
                                 start=(q == 0), stop=(q == 3))
            w_sb = elu_from_psum(wps[:], [L, 2 * D], "w", dt=BF16)
            wT = []
            for q, (o, s) in enumerate(CH2):
                dst = _t(work, [s, L], BF16, f"wT{q}")
                transpose_to(dst[:], w_sb[:, o:o + s], L, s,
                             slots=rot, si=q, dt=BF16)
                wT.append(dst)
            aps = _t(ps_mm, [L, 2 * D], F32, "mm", padded_shape=[L, 512])
            for q in range(4):
                nc.tensor.matmul(out=aps[:], lhsT=wT[q][:], rhs=Ws_sb[q][:],
                                 start=(q == 0), stop=(q == 3))
            atts_sb = _t(work, [L, 2 * D], BF16, "atts")
            nc.scalar.copy(atts_sb[:], aps[:])
            for q, (o, s) in enumerate(CH2):
                pool, tag = rot[q % len(rot)]
                aT = _t(pool, [s, L], BF16, tag, padded_shape=[s, 512])
                nc.tensor.transpose(out=aT[:, :],
                                    in_=atts_sb[:, o:o + s],
                                    identity=identb_sb[:, :])
                vT = _t(work, [s, L], F32, "vT")
                nc.vector.scalar_tensor_tensor(
                    vT[:], uT_list[q][:], 1.0, aT[:, :],
                    op0=ALU.mult, op1=ALU.mult,
                    accum_out=cv_sb[blk][0:s, q:q + 1])
            return mid_ret

        st_c = prep_block("c")
        load_late_weights()
        st_r = main_block("c", st_c, mid=lambda: prep_block("r"))
        main_block("r", st_r)

        # ---------- head: feat = [cv, rv, cv-rv, cv*rv]; y ----------
        diff = _t(singles, [L, 4], F32, "diff")
        nc.vector.tensor_sub(diff[:], cv_sb["c"][:], cv_sb["r"][:])
        prod = _t(singles, [L, 4], F32, "prod")
        nc.vector.tensor_mul(prod[:], cv_sb["c"][:], cv_sb["r"][:])
        groups_f = [cv_sb["c"], cv_sb["r"], diff, prod]
        groups = []
        for gi, g in enumerate(groups_f):
            gb = _t(singles, [L, 4], BF16, f"gb{gi}")
            nc.vector.tensor_copy(gb[:], g[:])
            groups.append(gb)

        y1A = _t(ps_st, [128, 1], F32, "S", padded_shape=[128, 512])
        y1B = _t(ps_st, [72, 1], F32, "T", padded_shape=[72, 512])
        nmm = 4 * len(CH2)
        kc = 0
        for g in range(4):
            for q, (o, s) in enumerate(CH2):
                col = groups[g][0:s, q:q + 1]
                nc.tensor.matmul(out=y1A[:], lhsT=F1_sb[kc][:, 0:128], rhs=col,
                                 start=(kc == 0), stop=(kc == nmm - 1))
                kc += 1
        kc = 0
        for g in range(4):
            for q, (o, s) in enumerate(CH2):
                col = groups[g][0:s, q:q + 1]
                nc.tensor.matmul(out=y1B[:], lhsT=F1_sb[kc][:, 128:200], rhs=col,
                                 start=(kc == 0), stop=(kc == nmm - 1))
                kc += 1
        r1A = _t(sml, [128, 1], BF16, "r1A")
        nc.scalar.activation(r1A[:], y1A[:], AF.Relu)
        r1B = _t(sml, [72, 1], BF16, "r1B")
        nc.scalar.activation(r1B[:], y1B[:], AF.Relu)
        yps = _t(ps_mm, [L, 2 * D], F32, "mm", padded_shape=[L, 512])[0:1, 0:1]
        nc.tensor.matmul(out=yps[:], lhsT=r1A[:], rhs=F2A_sb[:],
                         start=True, stop=False)
        nc.tensor.matmul(out=yps[:], lhsT=r1B[:], rhs=F2B_sb[:],
                         start=False, stop=True)
        y_sb = _t(sml, [1, 1], F32, "ysb")
        nc.scalar.copy(y_sb[:], yps[:])
        nc.sync.dma_start(out=y_out, in_=y_sb[:])

    nc.compile()
    return nc


def _build_masks(ids):
    """[128, 256] bf16: col 2i+0 = fw col for query i (keys m>i), 2i+1 = bw
    (m<i); pad keys and pad queries zero the column."""
    np1 = (ids != PAD).astype(np.float32)
    m = np.arange(L)
    fw = (m[:, None] > m[None, :]).astype(np.float32) * np1[:, None] * np1[None, :]
    bw = (m[:, None] < m[None, :]).astype(np.float32) * np1[:, None] * np1[None, :]
    out = np.empty((L, 2 * L), np.float32)
    out[:, 0::2] = fw
    out[:, 1::2] = bw
    return out.astype(ml_dtypes.bfloat16)


def make_in_maps(inputs):
    x1 = np.asarray(inputs["x1"]).astype(np.int64)
    x2 = np.asarray(inputs["x2"]).astype(np.int64)
    f32 = lambda k: np.ascontiguousarray(np.asarray(inputs[k], np.float32))
    bf16 = lambda k: np.ascontiguousarray(
        np.asarray(inputs[k], np.float32).astype(ml_dtypes.bfloat16))
    emb = f32("emb_w")
    shared = {
        "emb": emb,
        "Wh": f32("Wh_w"), "W1": f32("W1_w"), "W2": f32("W2_w"),
        "Wf1": bf16("Wf1_w"), "Wf2": f32("Wf2_w"),
        "Ws1": bf16("Ws1_w"), "Ws": bf16("Ws_w"),
        "F1": bf16("F1_w"), "F2": bf16("F2_w").reshape(D, 1),
        "b_rep": np.tile(f32("b").reshape(1, D), (L, 1)),
        "ident_f": np.eye(L, dtype=np.float32),
        "ident_b": np.eye(L, dtype=np.float32).astype(ml_dtypes.bfloat16),
    }
    in_maps = []
    for bidx in range(N_CORES):
        m = dict(shared)
        m["xc_idx"] = x1[bidx].reshape(L, 1).astype(np.int32)
        m["xr_idx"] = x2[bidx].reshape(L, 1).astype(np.int32)
        m["masks_c"] = _build_masks(x1[bidx])
        m["masks_r"] = _build_masks(x2[bidx])
        in_maps.append(m)
    return in_maps


_NC_CACHE = {}


def get_nc():
    if "nc" not in _NC_CACHE:
        _NC_CACHE["nc"] = build_nc()
    return _NC_CACHE["nc"]


def kernel(**inputs) -> np.ndarray:
    from concourse.bass_utils import run_bass_kernel_spmd
    nc = get_nc()
    in_maps = make_in_maps(inputs)
    res = run_bass_kernel_spmd(nc, in_maps, list(range(N_CORES)))
    y = np.array([np.asarray(res.results[i]["y"]).reshape(-1)[0]
                  for i in range(N_CORES)], dtype=np.float32)
    return y


# revision 61
# speedup vs baseline: 1.0351x; 1.0235x over previous
"""DiSAN Trainium2 Bass kernel — 8-core data parallel (one example per core).

Per-core layout (one batch example, both text blocks x1/x2):
  - m (key token, 128) on SBUF partitions; (i=query, d=feature) on free axis.
  - att pre-activation G[m, i, d] = h1[i,d]+b[d] + h2[m,d] is built on the
    TensorEngine: rank-1 broadcast of (h1+b) with a ones lhsT as an exact
    bf16 hi/lo pair accumulated in PSUM; h2 added by the Vector/GpSimd
    engines (alternating, to keep the DVE off the critical path).
  - ScalarEngine: A = tanh(G/c) fp32, z = exp(c*A) bf16 (Tanh/Exp share one
    activation-table set — no table switches). 16-query chunks amortize the
    per-instruction overhead.
  - TensorEngine: per-query "flipped" matmuls — lhsT = z (or z*h) slice
    [m=128, d-chunk], rhs = the query's fw/bw 0/1 bf16 mask columns
    [m=128, 2] — contract over m, producing the directional softmax sums S
    (denominator) and T (numerator) directly in transposed [d, (query,
    dir)] layout. Feature dim is chunked {128, 72} so the dominant
    stationary loads hit the fast-weight-load path (needs exactly 128
    stationary columns).
  - S and T use one PSUM bank per round each (round 0: mm/tp slots,
    round 1: S/T slots) so a round's vector post never reads a bank the
    next round's matmuls are writing (PE-write + DVE-read of one bank is
    a hardware collision). zh+S/T emission is software-pipelined one
    chunk late and the round post is spread over the next round's chunks.
  - s = T/S, with the reference's all-masked-row behavior reproduced
    exactly: S==0 => s = sum_m h[m,:]/128 (uniform softmax).
  - Fusion gate f, u, att_s and the final MLP run in bf16 (weights shipped
    as bf16 from the host, PE transposes in bf16); sigmoid is computed as
    0.5*tanh(0.5x)+0.5 to stay in the same activation-table set.

kernel(**inputs) takes the full unsharded inputs (as produced by
setup_inputs) and returns the full (8,) output; it shards batch across the
8 NeuronCores internally via run_bass_kernel_spmd.
"""

from contextlib import ExitStack

import numpy as np
import ml_dtypes

import concourse.bass as bass
import concourse.bacc as bacc
import concourse.tile as tile
from concourse import mybir

F32 = mybir.dt.float32
BF16 = mybir.dt.bfloat16
I32 = mybir.dt.int32
AF = mybir.ActivationFunctionType
ALU = mybir.AluOpType
AX = mybir.AxisListType

L = 128          # sequence length
D = 200          # feature dim
VOCAB = 32000
PAD = 1
N_CORES = 8
CHUNK_I = 16     # queries per G/z chunk
N_CHUNKS = L // CHUNK_I
C_VAL = 5.0
FQ = 2 * D       # 400 = one query pair worth of (i, d)

# feature-dim chunks: 128-col chunks get the PE fast-weight-load path
CH = ((0, 128), (128, 72))                       # D = 200
CH2 = ((0, 128), (128, 72), (200, 128), (328, 72))   # 2D = 400


def build_nc():
    nc = bacc.Bacc("TRN2", target_bir_lowering=False, debug=False)

    def din(name, shape, dt):
        return nc.dram_tensor(name, shape, dt, kind="ExternalInput").ap()

    x_idx_d = {"c": din("xc_idx", [L, 1], I32), "r": din("xr_idx", [L, 1], I32)}
    emb = din("emb", [VOCAB, D], F32)
    Wh = din("Wh", [D, D], F32)
    W1 = din("W1", [D, D], F32)
    W2 = din("W2", [D, D], F32)
    Wf1 = din("Wf1", [D, D], BF16)
    Wf2 = din("Wf2", [D, D], F32)
    Ws1 = din("Ws1", [2 * D, 2 * D], BF16)
    Ws = din("Ws", [2 * D, 2 * D], BF16)
    F1 = din("F1", [8 * D, D], BF16)
    F2 = din("F2", [D, 1], BF16)
    b_rep = din("b_rep", [L, D], F32)
    masks_d = {"c": din("masks_c", [L, 2 * L], BF16),
               "r": din("masks_r", [L, 2 * L], BF16)}
    ident_f = din("ident_f", [L, L], F32)
    ident_b = din("ident_b", [L, L], BF16)

    y_out = nc.dram_tensor("y", [1, 1], F32, kind="ExternalOutput").ap()

    with tile.TileContext(nc) as tc, ExitStack() as ctx:
        singles = ctx.enter_context(tc.tile_pool(name="singles", bufs=1))
        blockp = ctx.enter_context(tc.tile_pool(name="blockp", bufs=2))
        work = ctx.enter_context(tc.tile_pool(name="work", bufs=2))
        sml = ctx.enter_context(tc.tile_pool(name="sml", bufs=2))
        ps_hrep = ctx.enter_context(tc.tile_pool(name="ps_hrep", bufs=1, space="PSUM"))
        ps_st = ctx.enter_context(tc.tile_pool(name="ps_st", bufs=1, space="PSUM"))
        ps_mm = ctx.enter_context(tc.tile_pool(name="ps_mm", bufs=1, space="PSUM"))
        ps_tp = ctx.enter_context(tc.tile_pool(name="ps_tp", bufs=1, space="PSUM"))

        def _t(pool, shape, dt, tag, **kw):
            return pool.tile(shape, dt, name=tag, tag=tag, **kw)

        # DMA queue discipline: weights on sync, gathers/masks on gpsimd.
        # Nothing on scalar/vector — their compute queues are load-bearing.
        def load(ap_dram, shape, dt, tag=None, eng=None):
            t = _t(singles, shape, dt, tag)
            (eng or nc.sync).dma_start(out=t[:], in_=ap_dram)
            return t

        # ---- critical-path loads first: gathers + the h-chain weights ----
        gath = {}
        for blk in ("c", "r"):
            idx_sb = _t(sml, [L, 1], I32, "idx")
            nc.gpsimd.dma_start(out=idx_sb[:], in_=x_idx_d[blk])
            xemb = _t(sml, [L, D], F32, "xemb")
            nc.gpsimd.indirect_dma_start(
                out=xemb[:], out_offset=None, in_=emb,
                in_offset=bass.IndirectOffsetOnAxis(ap=idx_sb[:, :1], axis=0))
            gath[blk] = xemb

        identf_sb = load(ident_f, [L, L], F32, "idf")
        Wh_sb = [load(Wh[o:o + s, :], [s, D], F32, f"Wh{k}")
                 for k, (o, s) in enumerate(CH)]
        W1_sb = [load(W1[o:o + s, :], [s, D], F32, f"W1{k}")
                 for k, (o, s) in enumerate(CH)]
        W2_sb = [load(W2[o:o + s, :], [s, D], F32, f"W2{k}")
                 for k, (o, s) in enumerate(CH)]
        brep_sb = load(b_rep, [L, D], F32, "brep", eng=nc.gpsimd)
        mask_sb = {"c": load(masks_d["c"], [L, 2 * L], BF16, "mskc", eng=nc.gpsimd),
                   "r": load(masks_d["r"], [L, 2 * L], BF16, "mskr", eng=nc.gpsimd)}
        # later-use gate/head weights: tiles declared here, DMAs issued
        # after prep_c (sync queue is idle by then; keeps the critical
        # path DMAs in front)
        identb_sb = _t(singles, [L, L], BF16, "idb")
        Wf1_sb = [_t(singles, [s, D], BF16, f"Wg1{k}")
                  for k, (o, s) in enumerate(CH)]
        Wf2_sb = [_t(singles, [s, D], F32, f"Wg2{k}")
                  for k, (o, s) in enumerate(CH)]
        Ws1_sb = [_t(singles, [s, 2 * D], BF16, f"Ws1{k}")
                  for k, (o, s) in enumerate(CH2)]
        Ws_sb = [_t(singles, [s, 2 * D], BF16, f"Ws{k}")
                 for k, (o, s) in enumerate(CH2)]
        F1_sb = [_t(singles, [s, D], BF16, f"F1_{g}_{q}")
                 for g in range(4) for q, (o, s) in enumerate(CH2)]
        F2A_sb = _t(singles, [128, 1], BF16, "F2A")
        F2B_sb = _t(singles, [72, 1], BF16, "F2B")

        def load_late_weights():
            nc.sync.dma_start(out=identb_sb[:], in_=ident_b)
            for k, (o, s) in enumerate(CH):
                nc.sync.dma_start(out=Wf1_sb[k][:], in_=Wf1[o:o + s, :])
                nc.sync.dma_start(out=Wf2_sb[k][:], in_=Wf2[o:o + s, :])
            for q, (o, s) in enumerate(CH2):
                nc.sync.dma_start(out=Ws1_sb[q][:], in_=Ws1[o:o + s, :])
                nc.sync.dma_start(out=Ws_sb[q][:], in_=Ws[o:o + s, :])
            kc = 0
            for g in range(4):
                for q, (o, s) in enumerate(CH2):
                    nc.sync.dma_start(
                        out=F1_sb[kc][:],
                        in_=F1[g * 2 * D + o:g * 2 * D + o + s, :])
                    kc += 1
            nc.sync.dma_start(out=F2A_sb[:], in_=F2[0:128, :])
            nc.sync.dma_start(out=F2B_sb[:], in_=F2[128:200, :])

        ones2_bf = _t(singles, [2, L], BF16, "ones2bf")
        nc.vector.memset(ones2_bf[:], 1.0)

        cv_sb = {"c": _t(singles, [L, 4], F32, "cv"),
                 "r": _t(singles, [L, 4], F32, "rv")}

        TP_ONLY = ((ps_tp, "tp"),)
        TP_ROT = ((ps_tp, "tp"), (ps_st, "S"), (ps_st, "T"))

        def transpose_to(dst_ap, src_ap, n_par, n_free, slots=TP_ONLY, si=0,
                         dt=F32):
            pool, tag = slots[si % len(slots)]
            tp = _t(pool, [n_free, n_par], dt, tag, padded_shape=[n_free, 512])
            ident = identf_sb if dt == F32 else identb_sb
            nc.tensor.transpose(out=tp[:, :], in_=src_ap,
                                identity=ident[0:n_par, 0:n_par])
            nc.scalar.copy(dst_ap, tp[:, :])

        def transpose_ch(src_ap, n_par, n_free, tag, slots=TP_ONLY, si=0):
            dst = _t(work, [n_free, n_par], F32, tag)
            transpose_to(dst[:], src_ap, n_par, n_free, slots=slots, si=si)
            return dst

        def elu_from_psum(ps_ap, shape, tag, dt=F32):
            r = _t(work, shape, F32, "elur")
            nc.scalar.activation(r[:], ps_ap, AF.Relu)
            mn = _t(work, shape, F32, "elum")
            nc.vector.tensor_scalar_min(mn[:], ps_ap, 0.0)
            ex = _t(work, shape, F32, "elue")
            nc.scalar.activation(ex[:], mn[:], AF.Exp)
            o = _t(work, shape, dt, tag + "_o")
            nc.vector.scalar_tensor_tensor(o[:], r[:], -1.0, ex[:],
                                           op0=ALU.add, op1=ALU.add)
            return o

        def prep_block(blk):
            # ---------- h = elu(x @ Wh) (gather already issued) ----------
            xemb = gath[blk]

            # prep_c runs before any round work, so it may rotate its
            # transposes across the then-free tp/S psum slots; prep_r runs
            # mid-main where "S" belongs to round 1.
            pslots = (((ps_tp, "tp"), (ps_st, "S")) if blk == "c"
                      else TP_ONLY)
            veng = nc.vector
            xembT = [transpose_ch(xemb[:, o:o + s], L, s, f"xT{k}",
                                  slots=pslots, si=k)
                     for k, (o, s) in enumerate(CH)]
            hpre = _t(ps_hrep, [L, D], F32, "hrep0", padded_shape=[L, 1024])
            for k in range(2):
                nc.tensor.matmul(out=hpre[:], lhsT=xembT[k][:], rhs=Wh_sb[k][:],
                                 start=(k == 0), stop=(k == 1))
            h_sb = elu_from_psum(hpre[:], [L, D], "h")
            h_bf = _t(sml, [L, D], BF16, "hbf")
            veng.tensor_copy(h_bf[:], h_sb[:])

            hT = [transpose_ch(h_sb[:, o:o + s], L, s, f"hT{k}",
                               slots=pslots, si=k)
                  for k, (o, s) in enumerate(CH)]

            # ---------- h2 = h @ W2 and h1b = h @ W1 + b ----------
            h2ps = _t(ps_hrep, [L, D], F32, "hrep1", padded_shape=[L, 1024])
            for k in range(2):
                nc.tensor.matmul(out=h2ps[:], lhsT=hT[k][:], rhs=W2_sb[k][:],
                                 start=(k == 0), stop=(k == 1))
            h2_sb = _t(sml, [L, D], F32, "h2sb")
            nc.scalar.copy(h2_sb[:], h2ps[:])

            h1ps = _t(ps_hrep, [L, D], F32, "hrep0", padded_shape=[L, 1024])
            for k in range(2):
                nc.tensor.matmul(out=h1ps[:], lhsT=hT[k][:], rhs=W1_sb[k][:],
                                 start=(k == 0), stop=(k == 1))
            h1b = _t(sml, [L, D], F32, "h1b")
            nc.vector.tensor_add(h1b[:], h1ps[:], brep_sb[:])
            # exact bf16 hi/lo pair of h1+b, flattened to [2, 25600] via
            # sbuf->sbuf DMA (partition-major flatten)
            h1hi = _t(sml, [L, D], BF16, "h1hi")
            veng.tensor_copy(h1hi[:], h1b[:])
            h1rem = _t(sml, [L, D], F32, "h1rem")
            veng.tensor_sub(h1rem[:], h1b[:], h1hi[:])
            h1lo = _t(sml, [L, D], BF16, "h1lo")
            veng.tensor_copy(h1lo[:], h1rem[:])
            flathl = _t(blockp, [2, L * D], BF16, "flathl", bufs=1)
            nc.sync.dma_start(out=flathl[0:1, :], in_=h1hi[:])
            nc.gpsimd.dma_start(out=flathl[1:2, :], in_=h1lo[:])

            # HallT[:, ch] = sum_m h[m, d-chunk] as columns (fix rows)
            HallT = _t(sml, [L, 2], F32, "hallT")
            for k, (o, s) in enumerate(CH):
                nc.vector.tensor_reduce(out=HallT[0:s, k:k + 1], in_=hT[k][:],
                                        axis=AX.X, op=ALU.add)
            # dense h repeat via small sbuf->sbuf DMAs (engines stay free):
            # lets the zh multiply run dense bf16 at 2x instead of 1x.
            # slice-granular subtile deps let early chunks start after a
            # few slices.
            hrep16 = _t(sml, [L, CHUNK_I, D], BF16, "hrep16")
            for r16 in range(CHUNK_I):
                eng = nc.sync if r16 % 2 == 0 else nc.gpsimd
                eng.dma_start(out=hrep16[:, r16, :], in_=h_bf[:])
            return dict(h_sb=h_sb, h_bf=h_bf, hT=hT, h2_sb=h2_sb,
                        flathl=flathl, HallT=HallT, hrep16=hrep16)

        def main_block(blk, st_, mid=None):
            msk = mask_sb[blk]
            h_sb, h_bf, hT, h2_sb = (st_["h_sb"], st_["h_bf"], st_["hT"],
                                     st_["h2_sb"])
            flathl, HallT = st_["flathl"], st_["HallT"]
            hrep16 = st_["hrep16"]

            # ---------- main loop: G -> tanh -> exp -> zh -> S/T ----------
            # per round, S and T share ONE psum bank (S cols 0:256, T cols
            # 256:512); rounds use different banks so round r's vector post
            # never reads a bank round r+1's matmuls are writing (PE-write +
            # DVE-read of the same bank is a hardware collision).
            sT = {0: [_t(blockp, [s, L], BF16, f"sTf{c}")
                      for c, (o, s) in enumerate(CH)],
                  1: [_t(blockp, [s, L], BF16, f"sTb{c}")
                      for c, (o, s) in enumerate(CH)]}
            # ragged chunks: small leading chunks shorten the first-z latency
            CHUNKS = {0: ((0, 4), (4, 4), (8, 8), (16, 16), (32, 16), (48, 16)),
                      1: ((64, 16), (80, 16), (96, 8), (104, 8), (112, 8),
                          (120, 4), (124, 4))}
            # software-pipelined emission: each chunk's zh+S/T matmuls are
            # emitted one chunk late (so the vector queue never stalls on
            # exp), and each round's post ops are spread over the next
            # round's chunks (disjoint psum banks make that overlap safe).
            pending = []
            pend_zh = []

            def emit_zh_st(z_sb, i0, cn, S_ps, T_ps):
                zh_sb = _t(work, [L, CHUNK_I, D], BF16, "zh", bufs=2)
                zr = z_sb[:].rearrange("p (a d) -> p a d", d=D)
                nv = cn - 4 if cn >= 16 else cn
                nc.vector.tensor_mul(
                    zh_sb[:, 0:nv, :], zr[:, 0:nv, :], hrep16[:, 0:nv, :])
                if nv < cn:
                    nc.gpsimd.tensor_mul(
                        zh_sb[:, nv:cn, :], zr[:, nv:cn, :],
                        hrep16[:, nv:cn, :])
                zh_2d = zh_sb[:].rearrange("p a d -> p (a d)")
                for iq in range(cn):
                    i = i0 + iq
                    j = i % 64
                    pm = msk[:, 2 * i:2 * i + 2]
                    for c, (o, s) in enumerate(CH):
                        co = 128 * c + 2 * j
                        nc.tensor.matmul(
                            out=S_ps[0:s, co:co + 2],
                            lhsT=z_sb[:, iq * D + o:iq * D + o + s],
                            rhs=pm, start=True, stop=True)
                        nc.tensor.matmul(
                            out=T_ps[0:s, co:co + 2],
                            lhsT=zh_2d[:, iq * D + o:iq * D + o + s],
                            rhs=pm, start=True, stop=True)

            def make_post(rnd, S_ps, T_ps):
                # ---- s = (T + ind*HallT) / (S + 128*ind), as closures ----
                # garbage rows 72:128 of the ch1 halves are never read.
                st = {}

                def p1():
                    st["ind"] = _t(work, [L, 2 * L], F32, "ind", bufs=1)
                    nc.vector.tensor_scalar(out=st["ind"][:], in0=S_ps[:, :],
                                            scalar1=0.0, scalar2=None,
                                            op0=ALU.is_equal)

                def p2():
                    S1 = _t(work, [L, 2 * L], F32, "S1", bufs=1)
                    nc.vector.scalar_tensor_tensor(S1[:], st["ind"][:], 128.0,
                                                   S_ps[:, :],
                                                   op0=ALU.mult, op1=ALU.add)
                    st["Sinv"] = _t(work, [L, 2 * L], F32, "Sinv", bufs=1)
                    nc.vector.reciprocal(st["Sinv"][:], S1[:])

                def p3():
                    TH = _t(work, [L, 2, L], F32, "TH", bufs=1)
                    nc.vector.tensor_mul(
                        TH[:], st["ind"][:].rearrange("p (a d) -> p a d", d=L),
                        HallT[:].unsqueeze(2).to_broadcast([L, 2, L]))
                    st["T1"] = _t(work, [L, 2 * L], F32, "T1", bufs=1)
                    nc.vector.tensor_add(st["T1"][:], T_ps[:, :],
                                         TH[:].rearrange("p a d -> p (a d)"))

                def p4(dire):
                    for c, (o, s) in enumerate(CH):
                        sl = slice(128 * c + dire, 128 * c + 128, 2)
                        nc.vector.tensor_mul(
                            sT[dire][c][:, 64 * rnd:64 * rnd + 64],
                            st["T1"][0:s, sl], st["Sinv"][0:s, sl])

                return [p1, p2, p3, lambda: p4(0), lambda: p4(1)]

            for rnd in range(2):
                # slot choice per (block, round): a block's round-0 slots
                # must not be the ones the PREVIOUS block's gates use
                # ("mm"/"tp"), or round 0 stalls until those gates drain.
                if (rnd == 0) == (blk == "c"):
                    S_ps = _t(ps_mm, [L, 2 * L], F32, "mm")
                    T_ps = _t(ps_tp, [L, 2 * L], F32, "tp")
                else:
                    S_ps = _t(ps_st, [L, 2 * L], F32, "S")
                    T_ps = _t(ps_st, [L, 2 * L], F32, "T")
                for (i0, cn) in CHUNKS[rnd]:
                    # h1+b broadcast to all partitions: k=2 hi/lo pair matmul
                    G_sb = _t(work, [L, CHUNK_I * D], F32, "G", bufs=2)
                    for hh in range(cn // 4):
                        hrep = _t(ps_hrep, [L, 2, 512], F32, f"hrep{hh % 2}")
                        for q in range(2):
                            o = (i0 + (2 * hh + q) * 2) * D
                            nc.tensor.matmul(out=hrep[:, q, 0:FQ],
                                             lhsT=ones2_bf[:],
                                             rhs=flathl[:, o:o + FQ],
                                             start=True, stop=True)
                        nc.vector.tensor_add(
                            G_sb[:, hh * 4 * D:(hh + 1) * 4 * D].rearrange(
                                "p (a b d) -> p a b d", b=2, d=D),
                            hrep[:, :, 0:FQ].rearrange(
                                "p a (b d) -> p a b d", d=D),
                            h2_sb[:].unsqueeze(1).unsqueeze(1).to_broadcast(
                                [L, 2, 2, D]))
                    A_sb = _t(work, [L, CHUNK_I * D], F32, "A", bufs=1)
                    nc.scalar.activation(A_sb[:, 0:cn * D], G_sb[:, 0:cn * D],
                                         AF.Tanh, scale=1.0 / C_VAL)
                    z_sb = _t(work, [L, CHUNK_I * D], BF16, "z", bufs=2)
                    nc.scalar.activation(z_sb[:, 0:cn * D], A_sb[:, 0:cn * D],
                                         AF.Exp, scale=C_VAL)
                    # flush: previous chunk's zh+S/T, then up to two spread
                    # post ops of the previous round
                    while pend_zh:
                        pend_zh.pop(0)()
                    for _ in range(2):
                        if pending:
                            pending.pop(0)()
                    pend_zh.append(
                        lambda z_sb=z_sb, i0=i0, cn=cn, S=S_ps, T=T_ps:
                        emit_zh_st(z_sb, i0, cn, S, T))
                # round done: queue the post; it must follow the last
                # chunk's zh+st, which pend_zh ordering guarantees
                pending.extend(make_post(rnd, S_ps, T_ps))
            # drain whatever is left (last chunk + round-1 post)
            while pend_zh:
                pend_zh.pop(0)()
            while pending:
                pending.pop(0)()

            # issue the next block's prep here: its engine work interleaves
            # with this block's gates in the queues (both independent)
            mid_ret = mid() if mid is not None else None

            # ---------- fusion gate f, u (in transposed space) ----------
            # block r's tail is the exposed end of the kernel: rotate its
            # transposes/matmuls across the then-idle psum slots.
            # block c's u-math is latency-tolerant (cv_c needed only at the
            # head): run it on gpsimd so block r's G-adds aren't queued
            # behind it on the DVE.
            rot = TP_ROT if blk == "r" else TP_ONLY
            ueng = nc.gpsimd if blk == "c" else nc.vector
            uT = {}
            for dire in range(2):
                if blk == "r" and dire == 1:
                    fps = _t(ps_st, [L, D], F32, "S", padded_shape=[L, 512])
                else:
                    fps = _t(ps_mm, [L, D], F32, "mm", padded_shape=[L, 512])
                for k in range(2):
                    nc.tensor.matmul(out=fps[:], lhsT=sT[dire][k][:],
                                     rhs=Wf1_sb[k][:],
                                     start=(k == 0), stop=False)
                for k in range(2):
                    nc.tensor.matmul(out=fps[:], lhsT=hT[k][:],
                                     rhs=Wf2_sb[k][:],
                                     start=False, stop=(k == 1))
                tsig = _t(work, [L, D], BF16, "tsig")
                nc.scalar.activation(tsig[:], fps[:], AF.Tanh, scale=0.5)
                for c, (o, s) in enumerate(CH):
                    fT = _t(work, [s, L], F32, f"fT{dire}{c}")
                    transpose_to(fT[:], tsig[:, o:o + s], L, s,
                                 slots=rot, si=2 * dire + c, dt=BF16)
                    ueng.tensor_scalar(out=fT[:], in0=fT[:], scalar1=0.5,
                                        scalar2=0.5, op0=ALU.mult,
                                        op1=ALU.add)
                    # uT = sT + fT * (hT - sT)
                    dt_ = _t(work, [s, L], F32, f"d{dire}{c}")
                    ueng.tensor_sub(dt_[:], hT[c][:], sT[dire][c][:])
                    ueng.tensor_mul(dt_[:], fT[:], dt_[:])
                    u = _t(blockp, [s, L], BF16, f"uT{dire}{c}")
                    ueng.tensor_add(u[:], sT[dire][c][:], dt_[:])
                    uT[(dire, c)] = u
            uT_list = [uT[(0, 0)], uT[(0, 1)], uT[(1, 0)], uT[(1, 1)]]

            # ---------- att_s = elu(u @ Ws1) @ Ws ; cv = sum_i u*att_s ----------
            wps = _t(ps_mm, [L, 2 * D], F32, "mm", padded_shape=[L, 512])
            for q in range(4):
                nc.tensor.matmul(out=wps[:], lhsT=uT_list[q][:], rhs=Ws1_sb[q][:],
                                 start=(q == 0), stop=(q == 3))
            w_sb = elu_from_psum(wps[:], [L, 2 * D], "w", dt=BF16)
            wT = []
            for q, (o, s) in enumerate(CH2):
                dst = _t(work, [s, L], BF16, f"wT{q}")
                transpose_to(dst[:], w_sb[:, o:o + s], L, s,
                             slots=rot, si=q, dt=BF16)
                wT.append(dst)
            aps = _t(ps_mm, [L, 2 * D], F32, "mm", padded_shape=[L, 512])
            for q in range(4):
                nc.tensor.matmul(out=aps[:], lhsT=wT[q][:], rhs=Ws_sb[q][:],
                                 start=(q == 0), stop=(q == 3))
            atts_sb = _t(work, [L, 2 * D], BF16, "atts")
            nc.scalar.copy(atts_sb[:], aps[:])
            for q, (o, s) in enumerate(CH2):
                pool, tag = rot[q % len(rot)]
                aT = _t(pool, [s, L], BF16, tag, padded_shape=[s, 512])
                nc.tensor.transpose(out=aT[:, :],
                                    in_=atts_sb[:, o:o + s],
                                    identity=identb_sb[:, :])
                vT = _t(work, [s, L], F32, "vT")
                nc.vector.scalar_tensor_tensor(
                    vT[:], uT_list[q][:], 1.0, aT[:, :],
                    op0=ALU.mult, op1=ALU.mult,
                    accum_out=cv_sb[blk][0:s, q:q + 1])
            return mid_ret

        st_c = prep_block("c")
        load_late_weights()
        st_r = main_block("c", st_c, mid=lambda: prep_block("r"))
        main_block("r", st_r)

        # ---------- head: feat = [cv, rv, cv-rv, cv*rv]; y ----------
        diff = _t(singles, [L, 4], F32, "diff")
        nc.vector.tensor_sub(diff[:], cv_sb["c"][:], cv_sb["r"][:])
        prod = _t(singles, [L, 4], F32, "prod")
        nc.vector.tensor_mul(prod[:], cv_sb["c"][:], cv_sb["r"][:])
        groups_f = [cv_sb["c"], cv_sb["r"], diff, prod]
        groups = []
        for gi, g in enumerate(groups_f):
            gb = _t(singles, [L, 4], BF16, f"gb{gi}")
            nc.vector.tensor_copy(gb[:], g[:])
            groups.append(gb)

        y1A = _t(ps_st, [128, 1], F32, "S", padded_shape=[128, 512])
        y1B = _t(ps_st, [72, 1], F32, "T", padded_shape=[72, 512])
        nmm = 4 * len(CH2)
        kc = 0
        for g in range(4):
            for q, (o, s) in enumerate(CH2):
                col = groups[g][0:s, q:q + 1]
                nc.tensor.matmul(out=y1A[:], lhsT=F1_sb[kc][:, 0:128], rhs=col,
                                 start=(kc == 0), stop=(kc == nmm - 1))
                kc += 1
        kc = 0
        for g in range(4):
            for q, (o, s) in enumerate(CH2):
                col = groups[g][0:s, q:q + 1]
                nc.tensor.matmul(out=y1B[:], lhsT=F1_sb[kc][:, 128:200], rhs=col,
                                 start=(kc == 0), stop=(kc == nmm - 1))
                kc += 1
        r1A = _t(sml, [128, 1], BF16, "r1A")
        nc.scalar.activation(r1A[:], y1A[:], AF.Relu)
        r1B = _t(sml, [72, 1], BF16, "r1B")
        nc.scalar.activation(r1B[:], y1B[:], AF.Relu)
        yps = _t(ps_mm, [L, 2 * D], F32, "mm", padded_shape=[L, 512])[0:1, 0:1]
        nc.tensor.matmul(out=yps[:], lhsT=r1A[:], rhs=F2A_sb[:],
                         start=True, stop=False)
        nc.tensor.matmul(out=yps[:], lhsT=r1B[:], rhs=F2B_sb[:],
                         start=False, stop=True)
        y_sb = _t(sml, [1, 1], F32, "ysb")
        nc.scalar.copy(y_sb[:], yps[:])
        nc.sync.dma_start(out=y_out, in_=y_sb[:])

    nc.compile()
    return nc


def _build_masks(ids):
    """[128, 256] bf16: col 2i+0 = fw col for query i (keys m>i), 2i+1 = bw
    (m<i); pad keys and pad queries zero the column."""
    np1 = (ids != PAD).astype(np.float32)
    m = np.arange(L)
    fw = (m[:, None] > m[None, :]).astype(np.float32) * np1[:, None] * np1[None, :]
    bw = (m[:, None] < m[None, :]).astype(np.float32) * np1[:, None] * np1[None, :]
    out = np.empty((L, 2 * L), np.float32)
    out[:, 0::2] = fw
    out[:, 1::2] = bw
    return out.astype(ml_dtypes.bfloat16)


def make_in_maps(inputs):
    x1 = np.asarray(inputs["x1"]).astype(np.int64)
    x2 = np.asarray(inputs["x2"]).astype(np.int64)
    f32 = lambda k: np.ascontiguousarray(np.asarray(inputs[k], np.float32))
    bf16 = lambda k: np.ascontiguousarray(
        np.asarray(inputs[k], np.float32).astype(ml_dtypes.bfloat16))
    emb = f32("emb_w")
    shared = {
        "emb": emb,
        "Wh": f32("Wh_w"), "W1": f32("W1_w"), "W2": f32("W2_w"),
        "Wf1": bf16("Wf1_w"), "Wf2": f32("Wf2_w"),
        "Ws1": bf16("Ws1_w"), "Ws": bf16("Ws_w"),
        "F1": bf16("F1_w"), "F2": bf16("F2_w").reshape(D, 1),
        "b_rep": np.tile(f32("b").reshape(1, D), (L, 1)),
        "ident_f": np.eye(L, dtype=np.float32),
        "ident_b": np.eye(L, dtype=np.float32).astype(ml_dtypes.bfloat16),
    }
    in_maps = []
    for bidx in range(N_CORES):
        m = dict(shared)
        m["xc_idx"] = x1[bidx].reshape(L, 1).astype(np.int32)
        m["xr_idx"] = x2[bidx].reshape(L, 1).astype(np.int32)
        m["masks_c"] = _build_masks(x1[bidx])
        m["masks_r"] = _build_masks(x2[bidx])
        in_maps.append(m)
    return in_maps


_NC_CACHE = {}


def get_nc():
    if "nc" not in _NC_CACHE:
        _NC_CACHE["nc"] = build_nc()
    return _NC_CACHE["nc"]


def kernel(**inputs) -> np.ndarray:
    from concourse.bass_utils import run_bass_kernel_spmd
    nc = get_nc()
    in_maps = make_in_maps(inputs)
    res = run_bass_kernel_spmd(nc, in_maps, list(range(N_CORES)))
    y = np.array([np.asarray(res.results[i]["y"]).reshape(-1)[0]
                  for i in range(N_CORES)], dtype=np.float32)
    return y
